# revision 23
# baseline (speedup 1.0000x reference)
"""LlamaAttention (B=2, S=2048, H=4096, NH=32) on 8 Trainium2 NeuronCores.

Sharding: tensor-parallel over heads (4 heads / core). Column-parallel
Wq/Wk/Wv, row-parallel Wo; the Wo partial sums are reduced on the host
(the all-reduce of the TP recipe, done during unshard).

v2 dataflow (PE-roofline oriented):
  - Wq/Wk/Wv are cached in SBUF in bf16 for the whole of phase 1 (the
    v1 kernel re-streamed them every token slice: ~200 MB of HBM
    traffic became ~13 MB).
  - X^T streams through SBUF in bf16 chunks of 512 tokens.
  - Q^T/K^T (RoPE applied, fp32) go to DRAM scratch in fp32r so the
    phase-2 score matmuls keep fp32 precision; V goes in bf16.
  - The causal mask has only 4 distinct 128x512 diagonal block
    patterns; they are loaded once and reused (v1 streamed ~33 MB).
  - Softmax: exp on ACT (bf16 out), denominators via ones-matmul on PE,
    normalization on DVE. PSUM->SBUF copies ride the ACT engine.
  - 1/sqrt(HD) is folded into Wq on the host.
"""
import sys

sys.path.insert(0, "/opt/trn_rl_repo")

import numpy as np

import concourse.bass as bass
import concourse.bacc as bacc
import concourse.tile as tile
import concourse.mybir as mybir

B, S, H, NH = 2, 2048, 4096, 32
HD = H // NH          # 128
NC = 8                # cores
DL = H // NC          # 512 local dims (4 heads / core)
NHL = NH // NC        # 4 local heads
BT = B * S            # 4096 tokens
P = 128
CH = 512              # phase-1 token chunk (matmul moving dim)
QT = 512              # phase-2 query tile (free dim)
KT = 128              # phase-2 key tile (partition dim)
NKO = H // P          # 32 contraction subtiles

DT = mybir.dt.float32
DTR = mybir.dt.float32r
BF = mybir.dt.bfloat16
F32 = mybir.dt.float32
AF = mybir.ActivationFunctionType


def _phase1(nc, tc, pools, aps, scratches, pre_tiles):
    """QKV projections + RoPE, all batches, W cached in SBUF."""
    px, pt, p1r, pvo, psA, psV = pools
    xt3, wq3, wk3, wv3, cos_ap, sin_ap, wq_tiles, wk_tiles, wv_tiles = aps

    # Startup block in first-use order so the first V-wave matmul isn't
    # queued behind ~13 MB of weight cache.
    nc.sync.dma_start(wv_tiles[0][:], wv3[:, bass.ds(0, 8), :])
    xc0 = px.tile([P, NKO, CH], BF, tag="xt", name="xc0")
    nc.sync.dma_start(xc0[:], xt3[:, :, bass.ds(0, CH)])
    cos0 = pt.tile([P, CH], DT, tag="cos", name="cos0")
    nc.sync.dma_start(cos0[:], cos_ap[:, bass.ds(0, CH)])
    sin0 = pt.tile([P, CH], DT, tag="sin", name="sin0")
    nc.sync.dma_start(sin0[:], sin_ap[:, bass.ds(0, CH)])
    for g in range(1, 4):
        nc.sync.dma_start(wv_tiles[g][:], wv3[:, bass.ds(g * 8, 8), :])
    for dsub in range(DL // P):
        nc.sync.dma_start(wq_tiles[dsub][:], wq3[:, :, bass.ts(dsub, P)])
        nc.sync.dma_start(wk_tiles[dsub][:], wk3[:, :, bass.ts(dsub, P)])

    for c in range(BT // CH):                      # 8 chunks of 512 tokens
        b, pos = c // (S // CH), c % (S // CH)
        qt_d, kt_d, v_d = scratches[b]
        if c == 0:
            xc, cosT, sinT = xc0, cos0, sin0
        else:
            xc = px.tile([P, NKO, CH], BF, tag="xt")
            nc.sync.dma_start(xc[:], xt3[:, :, bass.ds(c * CH, CH)])
            cosT = pt.tile([P, CH], DT, tag="cos")
            nc.sync.dma_start(cosT[:], cos_ap[:, bass.ds(pos * CH, CH)])
            sinT = pt.tile([P, CH], DT, tag="sin")
            nc.sync.dma_start(sinT[:], sin_ap[:, bass.ds(pos * CH, CH)])

        # V first: its PSUM banks are recycled by phase-2 score tiles, so
        # their ACT copies must not be the last thing the chunk produces
        psums = [psV.tile([P, DL], F32, tag=f"v{j}", name=f"vps{j}")
                 for j in range(CH // P)]
        for hs in range(NKO):
            wv_sl = wv_tiles[hs // 8][:, hs % 8, :]
            for j in range(CH // P):
                nc.tensor.matmul(
                    psums[j][:], xc[:, hs, bass.ts(j, P)], wv_sl,
                    start=(hs == 0), stop=(hs == NKO - 1))
        for j in range(CH // P):
            vo = pvo.tile([P, DL], BF, tag="vo")
            nc.scalar.activation(vo[:], psums[j][:], AF.Copy)
            nc.sync.dma_start(
                v_d[bass.ds(pos * CH + j * P, P), :], vo[:])

        for (wt, outd) in ((wk_tiles, kt_d), (wq_tiles, qt_d)):
            for dsub in range(DL // P):
                psum = psA.tile([P, CH], F32, tag="qk")
                for hs in range(NKO):
                    nc.tensor.matmul(
                        psum[:], wt[dsub][:, hs, :], xc[:, hs, :],
                        start=(hs == 0), stop=(hs == NKO - 1))
                rc = p1r.tile([P, CH], DTR, tag="rc")
                rs = p1r.tile([P, CH], F32, tag="rs")
                nc.vector.tensor_mul(rc[:], psum[:], cosT[:])
                nc.vector.tensor_mul(
                    rs[0:64, :], psum[64:128, :], sinT[0:64, :])
                nc.vector.tensor_mul(
                    rs[64:128, :], psum[0:64, :], sinT[64:128, :])
                nc.vector.tensor_tensor(
                    rc[0:64, :], rc[0:64, :], rs[0:64, :],
                    mybir.AluOpType.subtract)
                nc.vector.tensor_tensor(
                    rc[64:128, :], rc[64:128, :], rs[64:128, :],
                    mybir.AluOpType.add)
                nc.sync.dma_start(
                    outd[bass.ts(dsub, P), bass.ds(pos * CH, CH)], rc[:])

        if c == (S // CH) - 1:
            # batch-0 scratch is complete: prefetch head 0 for phase 2 into
            # the dedicated (non-recycled) SBUF region, off the SP queue
            k0, q0, v0 = pre_tiles
            qt_d0, kt_d0, v_d0 = scratches[0]
            nc.gpsimd.dma_start(k0[:], kt_d0[bass.ds(0, P), :])
            nc.gpsimd.dma_start(q0[:], qt_d0[bass.ds(0, P), :])
            nc.gpsimd.dma_start(
                v0[:], v_d0[:, bass.ds(0, P)].rearrange("(kt p) d -> p kt d", p=P))


def _phase2_batch(nc, tc, b, spec, pools, mask_sb, maskt, mb, ones_bf,
                  scratch, ctxT, pre_tiles=None):
    """Attention for batch b -> ctxT [P, NHL, S]."""
    p2, p2e, p2m, psS, psSum, psC = pools
    qt_d, kt_d, v_d = scratch

    for h in range(NHL):
        if pre_tiles is not None and h == 0:
            k_sb, q_sb, v_sb = pre_tiles      # prefetched during phase 1
        else:
            # Pool-queue loads: prefetch out-of-band of the SP stream
            k_sb = p2.tile([P, S], DTR, tag="k_sb")
            nc.gpsimd.dma_start(k_sb[:], kt_d[bass.ts(h, P), :])
            q_sb = p2.tile([P, S], DTR, tag="q_sb")
            nc.gpsimd.dma_start(q_sb[:], qt_d[bass.ts(h, P), :])
            v_sb = p2.tile([P, S // P, P], BF, tag="v_sb")
            nc.gpsimd.dma_start(
                v_sb[:], v_d[:, bass.ts(h, P)].rearrange("(kt p) d -> p kt d", p=P))
        for qt in range(S // QT):
            blocks = spec[qt]
            nb = len(blocks)
            psum_sum = psSum.tile([P, QT], F32, tag="sum")
            psum_ctx = psC.tile([P, QT], F32, tag="ctx")
            for bi, (kt, pat) in enumerate(blocks):
                psum_s = psS.tile([P, QT], F32, tag="s")
                nc.tensor.matmul(
                    psum_s[:], k_sb[:, bass.ts(kt, KT)],
                    q_sb[:, bass.ts(qt, QT)], start=True, stop=True)
                if pat is not None and pat >= 0:
                    nc.vector.tensor_tensor(
                        psum_s[:], psum_s[:], mask_sb[:, pat, :],
                        mybir.AluOpType.add)
                elif pat is not None:          # general (non-causal) block
                    mk = p2m.tile([P, QT], DT, tag="mk")
                    nc.sync.dma_start(
                        mk[:], maskt[mb, bass.ts(kt, KT), bass.ts(qt, QT)])
                    nc.vector.tensor_tensor(
                        psum_s[:], psum_s[:], mk[:], mybir.AluOpType.add)
                e_sb = p2e.tile([P, QT], BF, tag="e")
                nc.scalar.activation(e_sb[:], psum_s[:], AF.Exp)
                nc.tensor.matmul(psum_sum[:], ones_bf[:], e_sb[:],
                                 start=(bi == 0), stop=(bi == nb - 1))
                nc.tensor.matmul(psum_ctx[:], v_sb[:, kt, :], e_sb[:],
                                 start=(bi == 0), stop=(bi == nb - 1))
            recip = p2e.tile([P, QT], F32, tag="recip")
            nc.vector.reciprocal(recip[:], psum_sum[:])
            nc.vector.tensor_mul(
                ctxT[:, h, bass.ts(qt, QT)], psum_ctx[:], recip[:])


def _phase3(nc, tc, pools, wo3, ctx_tiles, ot):
    p3w, p3o, psO = pools
    for oi in range(H // P):
        # one Wo tile serves both batches
        wo_sb = p3w.tile([P, NHL, P], DTR, tag="wo")
        nc.gpsimd.dma_start(wo_sb[:], wo3[:, :, bass.ts(oi, P)])
        for b in range(B):
            ctxT = ctx_tiles[b]
            for qt in range(S // QT):
                psum_o = psO.tile([P, QT], F32, tag="o")
                for hs in range(NHL):
                    nc.tensor.matmul(
                        psum_o[:], wo_sb[:, hs, :], ctxT[:, hs, bass.ts(qt, QT)],
                        start=(hs == 0), stop=(hs == NHL - 1))
                o_sb = p3o.tile([P, QT], DT, tag="o_sb")
                nc.scalar.activation(o_sb[:], psum_o[:], AF.Copy)
                nc.sync.dma_start(
                    ot[bass.ts(oi, P), bass.ds(b * S + qt * QT, QT)], o_sb[:])


def _build(specs, n_mb, reps=1, phases=(1, 2, 3), unroll=1):
    nc = bacc.Bacc()

    xt = nc.declare_dram_parameter("xt", [H, BT], BF, isOutput=False)
    wqt = nc.declare_dram_parameter("wqt", [H, DL], BF, isOutput=False)
    wkt = nc.declare_dram_parameter("wkt", [H, DL], BF, isOutput=False)
    wvt = nc.declare_dram_parameter("wvt", [H, DL], BF, isOutput=False)
    wot = nc.declare_dram_parameter("wot", [DL, H], DTR, isOutput=False)
    mask4 = nc.declare_dram_parameter("mask4", [4, KT, QT], DT, isOutput=False)
    maskt = nc.declare_dram_parameter("maskt", [n_mb, S, S], DT, isOutput=False)
    cos_p = nc.declare_dram_parameter("cos", [HD, S], DT, isOutput=False)
    sin_p = nc.declare_dram_parameter("sin", [HD, S], DT, isOutput=False)
    ot = nc.declare_dram_parameter("ot", [H, BT], DT, isOutput=True)

    xt3 = xt.rearrange("(ho p) t -> p ho t", p=P)
    wq3 = wqt.rearrange("(ho p) d -> p ho d", p=P)
    wk3 = wkt.rearrange("(ho p) d -> p ho d", p=P)
    wv3 = wvt.rearrange("(ho p) d -> p ho d", p=P)
    wo3 = wot.rearrange("(hs p) o -> p hs o", p=P)
    mask4r = mask4.rearrange("f p q -> p f q")

    import contextlib

    with tile.TileContext(nc) as tc:
        with (
            tc.tile_pool(name="glob", bufs=1) as glob,
            tc.tile_pool(name="dram", bufs=1, space="DRAM") as dram,
        ):
            scratches = []
            for b in range(B):
                qd = dram.tile([DL, S], DTR, tag=f"qt_d{b}", name=f"qt_d{b}")
                kd = dram.tile([DL, S], DTR, tag=f"kt_d{b}", name=f"kt_d{b}")
                vd = dram.tile([S, DL], BF, tag=f"v_d{b}", name=f"v_d{b}")
                scratches.append((qd, kd, vd))

            ones_f = glob.tile([P, P], F32, tag="ones_f")
            nc.any.memset(ones_f[:], 1.0)
            ones_bf = glob.tile([P, P], BF, tag="ones_bf")
            nc.vector.tensor_copy(ones_bf[:], ones_f[:])
            mask_sb = glob.tile([P, 4, QT], DT, tag="mask_sb")
            nc.gpsimd.dma_start(mask_sb[:], mask4r[:, :, :])
            # dedicated phase-2 head-0 landing zone (not recycled by the
            # phase-1 pools, so its DMA can run during late phase 1)
            k0_t = glob.tile([P, S], DTR, tag="k0_t")
            q0_t = glob.tile([P, S], DTR, tag="q0_t")
            v0_t = glob.tile([P, S // P, P], BF, tag="v0_t")
            pre_tiles = (k0_t, q0_t, v0_t)

            loop_cm = tc.For_i(0, reps, 1) if reps > 1 else contextlib.nullcontext()
            with loop_cm:
              for _un in range(unroll):
                if 1 in phases:
                    with (
                        tc.tile_pool(name="pw", bufs=1) as pw,
                        tc.tile_pool(name="px", bufs=2) as px,
                        tc.tile_pool(name="pt", bufs=2) as pt,
                        tc.tile_pool(name="p1r", bufs=2) as p1r,
                        tc.tile_pool(name="pvo", bufs=3) as pvo,
                        tc.tile_pool(name="psA", bufs=3, space="PSUM") as psA,
                        tc.tile_pool(name="psV", bufs=1, space="PSUM") as psV,
                    ):
                        wq_tiles = [pw.tile([P, NKO, P], BF, tag=f"wq{d}", name=f"wq{d}")
                                    for d in range(DL // P)]
                        wk_tiles = [pw.tile([P, NKO, P], BF, tag=f"wk{d}", name=f"wk{d}")
                                    for d in range(DL // P)]
                        wv_tiles = [pw.tile([P, 8, DL], BF, tag=f"wv{g}", name=f"wv{g}")
                                    for g in range(4)]
                        aps = (xt3, wq3, wk3, wv3, cos_p, sin_p,
                               wq_tiles, wk_tiles, wv_tiles)
                        _phase1(nc, tc, (px, pt, p1r, pvo, psA, psV),
                                aps, scratches, pre_tiles)
                if 2 in phases:
                    with tc.tile_pool(name="ctxp", bufs=1) as ctxp:
                        ctx_tiles = []
                        with (
                            tc.tile_pool(name="p2", bufs=3) as p2,
                            tc.tile_pool(name="p2e", bufs=3) as p2e,
                            tc.tile_pool(name="p2m", bufs=2) as p2m,
                            # psSum/psC first: they land on the phase-1 psA
                            # banks (drained late by DVE RoPE) but are only
                            # written after the first exp; psS gets the psV
                            # banks, freed early by the ACT V-copies.
                            tc.tile_pool(name="psSum", bufs=2, space="PSUM") as psSum,
                            tc.tile_pool(name="psC", bufs=2, space="PSUM") as psC,
                            tc.tile_pool(name="psS", bufs=4, space="PSUM") as psS,
                        ):
                            for b in range(B):
                                mb = b % n_mb
                                ctxT = ctxp.tile([P, NHL, S], DTR, tag=f"ctxT{b}",
                                                 name=f"ctxT{b}")
                                ctx_tiles.append(ctxT)
                                _phase2_batch(
                                    nc, tc, b, specs[mb],
                                    (p2, p2e, p2m, psS, psSum, psC),
                                    mask_sb, maskt, mb, ones_bf,
                                    scratches[b], ctxT,
                                    pre_tiles if (b == 0 and 1 in phases)
                                    else None)
                        if 3 in phases:
                            with (
                                tc.tile_pool(name="p3w", bufs=3) as p3w,
                                tc.tile_pool(name="p3o", bufs=6) as p3o,
                                tc.tile_pool(name="psO", bufs=6, space="PSUM") as psO,
                            ):
                                _phase3(nc, tc, (p3w, p3o, psO), wo3, ctx_tiles, ot)
    nc.finalize()
    return nc


def _rope_tables():
    inv_freq = 1.0 / (10000.0 ** (np.arange(0, HD, 2, dtype=np.float32) / HD))
    t = np.arange(S, dtype=np.float32)
    freqs = np.einsum("i,j->ij", t, inv_freq)
    emb = np.concatenate([freqs, freqs], axis=-1)        # [S, HD]
    return np.cos(emb).astype(np.float32), np.sin(emb).astype(np.float32)


def _block_spec(mask):
    """mask: [S, S] additive (q, k). Per-qt list of (kt, pat):
    pat None = unmasked block, 0..3 = cached causal diagonal pattern,
    -1 = general masked block (loaded from maskt)."""
    pats = [np.ascontiguousarray(mask[0:QT, kl * KT:(kl + 1) * KT].T)
            for kl in range(QT // KT)]
    spec = []
    for qt in range(S // QT):
        row = []
        sub_q = mask[qt * QT:(qt + 1) * QT]
        for kt in range(S // KT):
            blk = sub_q[:, kt * KT:(kt + 1) * KT]
            if np.all(blk <= -1e8):
                continue                        # fully masked -> skip
            if not np.any(blk != 0.0):
                row.append((kt, None))
                continue
            kl = kt - qt * (QT // KT)
            if 0 <= kl < QT // KT and np.array_equal(blk.T, pats[kl]):
                row.append((kt, kl))
            else:
                row.append((kt, -1))
        assert row, "a query tile with all keys masked is not supported"
        spec.append(row)
    return spec, pats


_CACHE = {}


def _prepare(hidden_states, attention_mask, Wq, Wk, Wv, Wo):
    """Host-side marshaling -> (specs, n_mb, in_maps)."""
    import ml_dtypes

    bf16 = ml_dtypes.bfloat16
    hidden_states = np.asarray(hidden_states, dtype=np.float32)
    attention_mask = np.asarray(attention_mask, dtype=np.float32)
    Wq = np.asarray(Wq, dtype=np.float32)
    Wk = np.asarray(Wk, dtype=np.float32)
    Wv = np.asarray(Wv, dtype=np.float32)
    Wo = np.asarray(Wo, dtype=np.float32)

    xt = np.ascontiguousarray(
        hidden_states.reshape(BT, H).T).astype(bf16)            # [H, BT]
    scale = 1.0 / np.sqrt(np.float32(HD))
    wqT = np.ascontiguousarray(Wq.T * scale).astype(bf16)       # [H, H] (in, out)
    wkT = np.ascontiguousarray(Wk.T).astype(bf16)
    wvT = np.ascontiguousarray(Wv.T).astype(bf16)
    woT = np.ascontiguousarray(Wo.T)                            # [H(in'), H(out)]

    masks = attention_mask[:, 0]                                # [B, S, S]
    same = bool(np.array_equal(masks[0], masks[1])) if B == 2 else True
    n_mb = 1 if same else B
    sp = [_block_spec(masks[i]) for i in range(n_mb)]
    specs = [s for s, _ in sp]
    mask4 = np.ascontiguousarray(np.stack(sp[0][1]))            # [4, KT, QT]
    maskt = np.ascontiguousarray(
        np.stack([masks[i].T for i in range(n_mb)]))            # [n_mb, S(k), S(q)]

    cos, sin = _rope_tables()
    cos_t = np.ascontiguousarray(cos.T)                         # [HD, S]
    sin_t = np.ascontiguousarray(sin.T)

    in_maps = []
    for g in range(NC):
        dsl = slice(g * DL, (g + 1) * DL)
        in_maps.append({
            "xt": xt,
            "wqt": np.ascontiguousarray(wqT[:, dsl]),
            "wkt": np.ascontiguousarray(wkT[:, dsl]),
            "wvt": np.ascontiguousarray(wvT[:, dsl]),
            "wot": np.ascontiguousarray(woT[dsl, :]),
            "mask4": mask4,
            "maskt": maskt,
            "cos": cos_t, "sin": sin_t,
        })
    return specs, n_mb, in_maps


def kernel(hidden_states, attention_mask, Wq, Wk, Wv, Wo):
    from concourse.bass_utils import run_bass_kernel_spmd

    specs, n_mb, in_maps = _prepare(
        hidden_states, attention_mask, Wq, Wk, Wv, Wo)

    key = (n_mb, tuple(tuple(map(tuple, s)) for s in specs))
    if key not in _CACHE:
        _CACHE[key] = _build(specs, n_mb)
    nc = _CACHE[key]

    try:
        res = run_bass_kernel_spmd(nc, in_maps, list(range(NC)), trace=False)
    except Exception:
        # one retry: a wedged NeuronCore usually recovers on re-dispatch
        import time as _time
        _time.sleep(5)
        res = run_bass_kernel_spmd(nc, in_maps, list(range(NC)), trace=False)
    acc = np.zeros((H, BT), dtype=np.float32)
    for g in range(NC):
        acc += res.results[g]["ot"]
    return np.ascontiguousarray(acc.T).reshape(B, S, H)
